# revision 57
# baseline (speedup 1.0000x reference)
"""PlainGCN message passing on 8 TRN2 NeuronCores.

Computation (reference):
    deg = bincount(h); dis = deg**-0.5; norm = dis[t]*dis[h]
    out = relu(segment_sum(norm[:,None] * x[h], t, N))

Strategy (v18, paired descriptors; v17 was ~200us, v13 231us):
  - Fold dis[h] into x host-side: x2 = dis[:,None]*x (bf16). Then
    out[t] = relu(dis[t] * segment_sum(x2[h], t)); dis[t] fused with
    the ReLU on ScalarE. Shard edges by destination: core c owns dest
    nodes [c*N/8, (c+1)*N/8).
  - The wall (v13-v17) is SWDGE descriptor generation: the Pool
    engine gather ucode is ~2.1 ns SERIAL per descriptor (4-queue
    batches; measured). Cost is per-DESCRIPTOR, not per-byte: 512B
    two-row descriptors cost ~2.12ns vs 2.01ns for 256B (measured)
    — so v18 makes most descriptors deliver TWO edge rows.
  - Per core, upload x2p: x2 rows permuted into stream-first-use
    order (distinct sources ~52.8k < 2*32767, so PAIR indices fit
    int16 over ONE window — no source buckets, one run per dest
    tile). Each tile's run puts first-occurrence slots first: their
    x2p rows are consecutive and even-aligned, so one elem_size=256
    descriptor covers TWO slots. Repeat slots (~30%) cost one
    descriptor each (used half = first-use parity, other half -1).
    Descriptors/core: ~76k -> ~50k.
  - Stream layout in PAIR units: runs back to back (run length = max
    over cores), end pad to 128 pairs. Stream slot s = (pair q=s/2,
    half h=s%2) -> gathered tile [128, cols, 256] at partition q%128,
    pair-col q/128, bytes h*256. Every pair slot is gathered (pads
    read row 0) — no memsets, no stale SBUF.
  - Segment-sum: one one-hot per (pair-col, half, tile) triple via
    is_equal(iota, tloc) in bf16, whole block in one DVE
    tensor_tensor (iota in PSUM: single SBUF port, doesn't stall Q7
    desc-gen). TensorE accumulates into PSUM; Relu(dis_t*psum) on
    ScalarE; DMA out. Last block's one-hots build up front.
  - 1024 (pair-)idxs is the hard ucode cap per dma_gather;
    single_packet=False; 4 SWDGE queues; deep FIFO + pools.
"""

import numpy as np

import concourse.bacc as bacc
import concourse.mybir as mybir
import concourse.tile as tile
from concourse.bass_utils import run_bass_kernel_spmd
from concourse.library_config import mlp as mlp_lib

P = 128
N_NODES = 100000
D_FEAT = 128
N_CORES = 8
TILE_BLOCK = 4       # dest tiles per block
GATHER_CHUNK = 1024  # max pair-idxs per dma_gather
GPOOL_BUFS = 14
OPOOL_BUFS = 3
N_SLICES = 4


def _preprocess(x, h, t):
    n, d = x.shape
    assert (n, d) == (N_NODES, D_FEAT)
    npc = n // N_CORES
    n_tiles = -(-npc // P)

    h = h.astype(np.int64)
    t = t.astype(np.int64)

    deg = np.bincount(h, minlength=n).astype(np.float64)
    dis = np.where(deg > 0, deg, 1.0) ** -0.5
    x2 = (x.astype(np.float64) * dis[:, None]).astype(np.float32)

    core = t // npc

    # Balanced node->tile assignment per core: flatten per-tile edge
    # counts so max-over-cores run lengths stay near the mean. Last
    # two blocks biased light so the stream tail is small.
    tloc_orig = t - core * npc
    newloc_of = np.zeros(n, dtype=np.int64)
    for c in range(N_CORES):
        base = c * npc
        vdeg = np.zeros(npc, dtype=np.int64)
        selc = np.nonzero(core == c)[0]
        np.add.at(vdeg, tloc_orig[selc], 1)
        order = np.argsort(-vdeg, kind="stable")
        loads = np.zeros(n_tiles, dtype=np.int64)
        slots = np.zeros(n_tiles, dtype=np.int64)
        fill = np.zeros(n_tiles, dtype=np.int64)
        cap = np.full(n_tiles, P, dtype=np.int64)
        cap[n_tiles - 1] = npc - (n_tiles - 1) * P
        bias = np.zeros(n_tiles, dtype=np.int64)
        lastblk0 = ((n_tiles - 1) // TILE_BLOCK) * TILE_BLOCK
        bias[lastblk0:] = 64
        if lastblk0 >= TILE_BLOCK:
            bias[lastblk0 - TILE_BLOCK:lastblk0] = 32
        newpos = np.zeros(npc, dtype=np.int64)
        for i in order:
            score = loads + bias
            score[slots >= cap] = 1 << 60
            jj_pick = int(np.argmin(score))
            loads[jj_pick] += vdeg[i]
            newpos[i] = jj_pick * P + fill[jj_pick]
            fill[jj_pick] += 1
            slots[jj_pick] += 1
        newloc_of[base:base + npc] = newpos

    tloc = newloc_of[t]
    j = tloc // P
    tin = (tloc % P).astype(np.float64)

    # ---- per-core stream construction (pair units) ----
    # Pass 1: per core, order edges by (tile, is_repeat, orig pos);
    # compute first-use perm with per-run even alignment; count pairs.
    per_core_raw = []
    pairs_per_run = np.zeros((N_CORES, n_tiles), dtype=np.int64)
    tbl_rows = np.zeros(N_CORES, dtype=np.int64)
    for c in range(N_CORES):
        sel = np.nonzero(core == c)[0]
        sel = sel[np.argsort(j[sel], kind="stable")]  # tile-major
        hs = h[sel]
        js = j[sel]
        # first occurrence of each source in this stream order
        _, first_idx = np.unique(hs, return_index=True)
        is_first = np.zeros(len(sel), dtype=bool)
        is_first[first_idx] = True
        # reorder within run: firsts first (stable)
        o2 = np.argsort(js * 2 + (~is_first), kind="stable")
        hs, js, is_first = hs[o2], js[o2], is_first[o2]
        sel = sel[o2]
        # per-run first/repeat counts
        f_cnt = np.bincount(js[is_first], minlength=n_tiles)
        r_cnt = np.bincount(js[~is_first], minlength=n_tiles)
        pairs_per_run[c] = (f_cnt + 1) // 2 + r_cnt
        # x2p row of each first (per-run even-aligned cumsum)
        run_base = np.zeros(n_tiles + 1, dtype=np.int64)
        for jj in range(n_tiles):
            run_base[jj + 1] = run_base[jj] + f_cnt[jj] + (f_cnt[jj] & 1)
        tbl_rows[c] = run_base[n_tiles]
        # perm[source] = x2p row (within-run rank of firsts)
        firsts_pos = np.nonzero(is_first)[0]
        within = np.zeros(len(firsts_pos), dtype=np.int64)
        prev = -1
        k = 0
        for idx3 in range(len(firsts_pos)):
            jj = js[firsts_pos[idx3]]
            if jj != prev:
                k = 0
                prev = jj
            within[idx3] = k
            k += 1
        perm = np.full(n, -1, dtype=np.int64)
        perm[hs[firsts_pos]] = run_base[js[firsts_pos]] + within
        per_core_raw.append((sel, hs, js, is_first, f_cnt, r_cnt,
                             run_base, perm))

    run_pairs = pairs_per_run.max(axis=0)  # shared schedule
    n_blocks = -(-n_tiles // TILE_BLOCK)
    run_start = np.zeros(n_tiles, dtype=np.int64)  # in pairs
    pos = 0
    for jj in range(n_tiles):
        run_start[jj] = pos
        pos += int(run_pairs[jj])
    e_pairs = -(-pos // P) * P
    n_pcols = e_pairs // P
    NTBL = int(tbl_rows.max())
    NTBL += NTBL & 1
    n_tblpairs = NTBL // 2
    assert n_tblpairs <= 32767

    # one-hot columns: mmcol = 2*pc + h per (pair-col, half, tile)
    ohcols = []
    tile_ohcols = [[] for _ in range(n_tiles)]
    blk_ohranges = []
    for blk in range(n_blocks):
        oh0 = len(ohcols)
        for jj in range(blk * TILE_BLOCK,
                        min((blk + 1) * TILE_BLOCK, n_tiles)):
            s, r = int(run_start[jj]), int(run_pairs[jj])
            if r == 0:
                continue
            for pc in range(s // P, (s + r - 1) // P + 1):
                for hh in (0, 1):
                    tile_ohcols[jj].append((len(ohcols), 2 * pc + hh))
                    ohcols.append((pc, hh, jj))
        blk_ohranges.append((oh0, len(ohcols)))
    n_oh = len(ohcols)
    max_blk_oh = max(b1 - b0 for (b0, b1) in blk_ohranges)

    first_blk_of_pcol = np.full(n_pcols, n_blocks, dtype=np.int64)
    for (pc, hh, jj) in ohcols:
        first_blk_of_pcol[pc] = min(first_blk_of_pcol[pc],
                                    jj // TILE_BLOCK)

    # gidx slices + gather chunks over pair space
    slices = []
    per = -(-(e_pairs // N_SLICES) // P) * P
    lo = 0
    while lo < e_pairs:
        hi = min(lo + per, e_pairs)
        slices.append((lo, hi))
        lo = hi
    gathers = []  # (slice_k, s0, ln, fc) in pairs
    for k, (lo, hi) in enumerate(slices):
        c0 = lo
        while c0 < hi:
            ln = min(GATHER_CHUNK, hi - c0)
            fc = int(first_blk_of_pcol[c0 // P:(c0 + ln) // P].min())
            gathers.append((k, c0, ln, fc))
            c0 += ln
    gathers.sort(key=lambda g: (g[3], g[1]))

    # ---- per-core tensors ----
    import ml_dtypes
    per_core = []
    for c in range(N_CORES):
        (sel, hs, js, is_first, f_cnt, r_cnt, run_base,
         perm) = per_core_raw[c]
        # pair idx + slot meta
        gi = np.zeros(e_pairs, dtype=np.int16)          # pair idx
        tf = np.full(2 * e_pairs, -1.0, dtype=np.float64)
        town = np.full(2 * e_pairs, -1, dtype=np.int64)
        tin_c = tin[sel]
        # firsts: per run, slot k -> pair run_start+k/2, half k%2,
        # x2p row run_base+k
        fpos = np.nonzero(is_first)[0]
        rpos = np.nonzero(~is_first)[0]
        # within-run ranks
        def ranks(pos_arr, runs_arr):
            out = np.zeros(len(pos_arr), dtype=np.int64)
            prev = -1
            k = 0
            for i3 in range(len(pos_arr)):
                jj = runs_arr[i3]
                if jj != prev:
                    k = 0
                    prev = jj
                out[i3] = k
                k += 1
            return out
        frank = ranks(fpos, js[fpos])
        rrank = ranks(rpos, js[rpos])
        # firsts
        q_f = run_start[js[fpos]] + frank // 2
        h_f = frank % 2
        slot_f = 2 * q_f + h_f
        tf[slot_f] = tin_c[fpos]
        town[slot_f] = js[fpos]
        # pair idx for first-pairs: row/2 where row = run_base + 2*(pair offset)
        gi[q_f] = ((run_base[js[fpos]] + frank) // 2).astype(np.int16)
        # repeats: pair slot after the first-pairs of the run
        fp = (f_cnt + 1) // 2
        q_r = run_start[js[rpos]] + fp[js[rpos]] + rrank
        p_r = perm[hs[rpos]]
        h_r = p_r % 2
        slot_r = 2 * q_r + h_r
        tf[slot_r] = tin_c[rpos]
        town[slot_r] = js[rpos]
        gi[q_r] = (p_r // 2).astype(np.int16)

        # x2p table
        x2p = np.zeros((NTBL, d), dtype=np.float64)
        used = perm >= 0
        x2p[perm[used]] = x2[np.nonzero(used)[0]]
        x2p = x2p.reshape(NTBL // 2, 2 * d).astype(ml_dtypes.bfloat16)

        # wrap pair idx: [16, e/16] tiled x8 -> [128, e/16]
        wrap = np.tile(gi.reshape(e_pairs // 16, 16).T,
                       (8, 1)).astype(np.int16)

        # meta[p, k] for ohcol k=(pc, hh, tile)
        tf3 = tf.reshape(n_pcols, P, 2)      # [pc, partition, half]
        town3 = town.reshape(n_pcols, P, 2)
        pcarr = np.array([pc for (pc, _h, _j) in ohcols])
        hharr = np.array([_h for (_pc, _h, _j) in ohcols])
        jarr = np.array([_j for (_pc, _h, _j) in ohcols])
        m3 = np.where(town3[pcarr, :, hharr] == jarr[:, None],
                      tf3[pcarr, :, hharr], -1.0)  # [n_oh, 128]
        meta = m3.T.astype(ml_dtypes.bfloat16).copy()

        dnode = np.zeros(n_tiles * P, dtype=np.float32)
        nl = newloc_of[c * npc:(c + 1) * npc]
        dnode[nl] = dis[c * npc:(c + 1) * npc].astype(np.float32)
        dis_t = dnode.reshape(n_tiles, P).T.copy()

        per_core.append({"gidx": wrap, "meta": meta, "dis": dis_t,
                         "x2p": x2p})

    iota = np.tile(np.arange(P, dtype=np.float32), (P, 1))

    sched = {
        "n": n, "d": d, "npc": npc, "n_tiles": n_tiles,
        "e_pairs": e_pairs, "n_tblpairs": n_tblpairs,
        "n_blocks": n_blocks, "gathers": gathers, "slices": slices,
        "ohcols": ohcols, "tile_ohcols": tile_ohcols,
        "blk_ohranges": blk_ohranges, "n_oh": n_oh,
        "max_blk_oh": max_blk_oh, "newloc_of": newloc_of,
    }
    return sched, per_core, iota


def _build_program(sched):
    n, d, npc = sched["n"], sched["d"], sched["npc"]
    n_tiles, e_pairs = sched["n_tiles"], sched["e_pairs"]
    n_tblpairs = sched["n_tblpairs"]
    n_blocks, gathers = sched["n_blocks"], sched["gathers"]
    tile_ohcols = sched["tile_ohcols"]
    blk_ohranges = sched["blk_ohranges"]
    n_oh, max_blk_oh = sched["n_oh"], sched["max_blk_oh"]
    slices = sched["slices"]

    nc = bacc.Bacc("TRN2", target_bir_lowering=False, debug=False,
                   num_devices=N_CORES, num_swdge_queues=4,
                   dynamic_dma_scratch_size=98304)
    f32 = mybir.dt.float32
    bf16 = mybir.dt.bfloat16
    x_d = nc.dram_tensor("x2p", [n_tblpairs, 2 * d], bf16,
                         kind="ExternalInput")
    iota_d = nc.dram_tensor("iota", [P, P], f32, kind="ExternalInput")
    gidx_d = nc.dram_tensor("gidx", [P, e_pairs // 16], mybir.dt.int16,
                            kind="ExternalInput")
    meta_d = nc.dram_tensor("meta", [P, n_oh], bf16, kind="ExternalInput")
    dis_d = nc.dram_tensor("dis", [P, n_tiles], f32, kind="ExternalInput")
    y_d = nc.dram_tensor("y", [npc, d], f32, kind="ExternalOutput")

    nc.gpsimd.load_library(mlp_lib)

    gather_of_pcol = {}
    for gid, (sk, s0, ln, _fc) in enumerate(gathers):
        for pc in range(s0 // P, (s0 + ln) // P):
            gather_of_pcol[pc] = (gid, s0 // P)

    relu = mybir.ActivationFunctionType.Relu

    with tile.TileContext(nc) as tc:
        with (
            tc.tile_pool(name="const", bufs=1) as cpool,
            tc.tile_pool(name="gather", bufs=GPOOL_BUFS) as gpool,
            tc.tile_pool(name="onehot", bufs=OPOOL_BUFS) as opool,
            tc.tile_pool(name="psum", bufs=7, space="PSUM") as ppool,
            tc.tile_pool(name="psiota", bufs=1, space="PSUM") as ipool,
            tc.tile_pool(name="outs", bufs=6) as ypool,
        ):
            gidx_tiles = {}
            for k, (s0, s1) in enumerate(slices):
                gix = cpool.tile([P, (s1 - s0) // 16], mybir.dt.int16,
                                 tag=f"gidx{k}")
                nc.sync.dma_start(gix[:], gidx_d[:, s0 // 16:s1 // 16])
                gidx_tiles[k] = (gix, s0)
                if k == 0:
                    iota_t = cpool.tile([P, P], f32, tag="iota")
                    nc.sync.dma_start(iota_t[:], iota_d[:, :])
                    meta_t = cpool.tile([P, n_oh], bf16, tag="meta")
                    nc.sync.dma_start(meta_t[:], meta_d[:, :])
                    dis_t = cpool.tile([P, n_tiles], f32, tag="dis")
                    nc.sync.dma_start(dis_t[:], dis_d[:, :])

            iota_p = ipool.tile([P, P], f32, tag="iop")
            nc.vector.tensor_copy(iota_p[:], iota_t[:])

            # last block's one-hots depend only on meta: build up front
            lo0, lo1 = blk_ohranges[n_blocks - 1]
            nlo = lo1 - lo0
            last_oh = cpool.tile([P, nlo * P], mybir.dt.float8e4, tag="lastoh")
            nc.vector.tensor_tensor(
                last_oh[:, :nlo * P].rearrange("p (c f) -> p c f", f=P),
                iota_p[:, None, :].broadcast_to([P, nlo, P]),
                meta_t[:, lo0:lo1, None].broadcast_to([P, nlo, P]),
                mybir.AluOpType.is_equal,
            )

            gtiles = {}

            def issue_gather(gid):
                sk, s0, ln, _fc = gathers[gid]
                gt = gpool.tile([P, (GATHER_CHUNK // P) * 2 * d], bf16,
                                tag="gt", name=f"gt{gid}")
                ncols_g = ln // P
                gt_3d = gt[:, :ncols_g * 2 * d].rearrange(
                    "p (c d) -> p c d", d=2 * d)
                gix, sl0 = gidx_tiles[sk]
                nc.gpsimd.dma_gather(
                    gt_3d,
                    x_d[:, :],
                    gix[:, (s0 - sl0) // 16:(s0 + ln - sl0) // 16],
                    ln, ln, 2 * d,
                    single_packet=False,
                    queue_num=gid % 4,
                )
                gtiles[gid] = gt

            next_gather = 0
            for blk in range(n_blocks):
                tiles_blk = range(blk * TILE_BLOCK,
                                  min((blk + 1) * TILE_BLOCK, n_tiles))
                while next_gather < len(gathers):
                    if gathers[next_gather][3] > blk + 1:
                        break
                    issue_gather(next_gather)
                    next_gather += 1

                oh0, oh1 = blk_ohranges[blk]
                nbo = oh1 - oh0
                if blk == n_blocks - 1:
                    ohblk = last_oh
                else:
                    ohblk = opool.tile([P, max_blk_oh * P], mybir.dt.float8e4,
                                       tag="ohb", name=f"ohb{blk}")
                    nc.vector.tensor_tensor(
                        ohblk[:, :nbo * P].rearrange(
                            "p (c f) -> p c f", f=P),
                        iota_p[:, None, :].broadcast_to([P, nbo, P]),
                        meta_t[:, oh0:oh1, None].broadcast_to([P, nbo, P]),
                        mybir.AluOpType.is_equal,
                    )

                for jj in tiles_blk:
                    ohlist = tile_ohcols[jj]
                    rows = min(P, npc - jj * P)
                    yt = ypool.tile([P, d], f32, tag="yt", name=f"yt{jj}")
                    pt = ppool.tile([P, d], f32, tag="ps", name=f"ps{jj}")
                    for si, (ohk, col2) in enumerate(ohlist):
                        pc, hh = col2 // 2, col2 % 2
                        gid, pc0 = gather_of_pcol[pc]
                        gt = gtiles[gid]
                        off = (pc - pc0) * 2 * d + hh * d
                        nc.tensor.matmul(
                            pt[:],
                            lhsT=ohblk[:, (ohk - oh0) * P:
                                       (ohk - oh0 + 1) * P],
                            rhs=gt[:, off:off + d],
                            start=(si == 0),
                            stop=(si == len(ohlist) - 1),
                        )
                    nc.scalar.activation(yt[:], pt[:], relu,
                                         scale=dis_t[:, jj:jj + 1])
                    nc.sync.dma_start(y_d[jj * P:jj * P + rows, :],
                                      yt[:rows, :])

    nc.compile()
    return nc


def _run(x, h, t, trace=False):
    import time
    t0 = time.monotonic()
    sched, per_core, iota = _preprocess(np.asarray(x), np.asarray(h),
                                        np.asarray(t))
    t1 = time.monotonic()
    print(f"[kernel] preprocess {t1 - t0:.1f}s  e_pairs={sched['e_pairs']} "
          f"pcols={sched['e_pairs'] // P} oh={sched['n_oh']} "
          f"tblpairs={sched['n_tblpairs']} "
          f"gathers={len(sched['gathers'])}", flush=True)
    nc = _build_program(sched)
    t2 = time.monotonic()
    print(f"[kernel] build {t2 - t1:.1f}s", flush=True)
    in_maps = [
        {"x2p": pc["x2p"], "iota": iota, "gidx": pc["gidx"],
         "meta": pc["meta"], "dis": pc["dis"]}
        for pc in per_core
    ]
    res = run_bass_kernel_spmd(nc, in_maps, core_ids=list(range(N_CORES)),
                               trace=trace)
    t3 = time.monotonic()
    print(f"[kernel] compile+run {t3 - t2:.1f}s", flush=True)
    ycat = np.concatenate([res.results[c]["y"] for c in range(N_CORES)],
                          axis=0)
    npc = sched["npc"]
    newloc = sched["newloc_of"]
    src = (np.arange(len(newloc)) // npc) * npc + newloc
    y = ycat[src]
    return y, res


def kernel(x, h, t):
    y, _ = _run(np.asarray(x), np.asarray(h), np.asarray(t))
    return y


# revision 58
# speedup vs baseline: 1.0202x; 1.0202x over previous
"""PlainGCN message passing on 8 TRN2 NeuronCores.

Computation (reference):
    deg = bincount(h); dis = deg**-0.5; norm = dis[t]*dis[h]
    out = relu(segment_sum(norm[:,None] * x[h], t, N))

Strategy (v18, paired descriptors; v17 was ~200us, v13 231us):
  - Fold dis[h] into x host-side: x2 = dis[:,None]*x (bf16). Then
    out[t] = relu(dis[t] * segment_sum(x2[h], t)); dis[t] fused with
    the ReLU on ScalarE. Shard edges by destination: core c owns dest
    nodes [c*N/8, (c+1)*N/8).
  - The wall (v13-v17) is SWDGE descriptor generation: the Pool
    engine gather ucode is ~2.1 ns SERIAL per descriptor (4-queue
    batches; measured). Cost is per-DESCRIPTOR, not per-byte: 512B
    two-row descriptors cost ~2.12ns vs 2.01ns for 256B (measured)
    — so v18 makes most descriptors deliver TWO edge rows.
  - Per core, upload x2p: x2 rows permuted into stream-first-use
    order (distinct sources ~52.8k < 2*32767, so PAIR indices fit
    int16 over ONE window — no source buckets, one run per dest
    tile). Each tile's run puts first-occurrence slots first: their
    x2p rows are consecutive and even-aligned, so one elem_size=256
    descriptor covers TWO slots. Repeat slots (~30%) cost one
    descriptor each (used half = first-use parity, other half -1).
    Descriptors/core: ~76k -> ~50k.
  - Stream layout in PAIR units: runs back to back (run length = max
    over cores), end pad to 128 pairs. Stream slot s = (pair q=s/2,
    half h=s%2) -> gathered tile [128, cols, 256] at partition q%128,
    pair-col q/128, bytes h*256. Every pair slot is gathered (pads
    read row 0) — no memsets, no stale SBUF.
  - Segment-sum: one one-hot per (pair-col, half, tile) triple via
    is_equal(iota, tloc) in bf16, whole block in one DVE
    tensor_tensor (iota in PSUM: single SBUF port, doesn't stall Q7
    desc-gen). TensorE accumulates into PSUM; Relu(dis_t*psum) on
    ScalarE; DMA out. Last block's one-hots build up front.
  - 1024 (pair-)idxs is the hard ucode cap per dma_gather;
    single_packet=False; 4 SWDGE queues; deep FIFO + pools.
"""

import numpy as np

import concourse.bacc as bacc
import concourse.mybir as mybir
import concourse.tile as tile
from concourse.bass_utils import run_bass_kernel_spmd
from concourse.library_config import mlp as mlp_lib

P = 128
N_NODES = 100000
D_FEAT = 128
N_CORES = 8
TILE_BLOCK = 4       # dest tiles per block
GATHER_CHUNK = 1024  # max pair-idxs per dma_gather
GPOOL_BUFS = 16
OPOOL_BUFS = 4
N_SLICES = 4


def _preprocess(x, h, t):
    n, d = x.shape
    assert (n, d) == (N_NODES, D_FEAT)
    npc = n // N_CORES
    n_tiles = -(-npc // P)

    h = h.astype(np.int64)
    t = t.astype(np.int64)

    deg = np.bincount(h, minlength=n).astype(np.float64)
    dis = np.where(deg > 0, deg, 1.0) ** -0.5
    x2 = (x.astype(np.float64) * dis[:, None]).astype(np.float32)

    core = t // npc

    # Balanced node->tile assignment per core: flatten per-tile edge
    # counts so max-over-cores run lengths stay near the mean. Last
    # two blocks biased light so the stream tail is small.
    tloc_orig = t - core * npc
    newloc_of = np.zeros(n, dtype=np.int64)
    for c in range(N_CORES):
        base = c * npc
        vdeg = np.zeros(npc, dtype=np.int64)
        selc = np.nonzero(core == c)[0]
        np.add.at(vdeg, tloc_orig[selc], 1)
        order = np.argsort(-vdeg, kind="stable")
        loads = np.zeros(n_tiles, dtype=np.int64)
        slots = np.zeros(n_tiles, dtype=np.int64)
        fill = np.zeros(n_tiles, dtype=np.int64)
        cap = np.full(n_tiles, P, dtype=np.int64)
        cap[n_tiles - 1] = npc - (n_tiles - 1) * P
        bias = np.zeros(n_tiles, dtype=np.int64)
        lastblk0 = ((n_tiles - 1) // TILE_BLOCK) * TILE_BLOCK
        bias[lastblk0:] = 64
        if lastblk0 >= TILE_BLOCK:
            bias[lastblk0 - TILE_BLOCK:lastblk0] = 32
        newpos = np.zeros(npc, dtype=np.int64)
        for i in order:
            score = loads + bias
            score[slots >= cap] = 1 << 60
            jj_pick = int(np.argmin(score))
            loads[jj_pick] += vdeg[i]
            newpos[i] = jj_pick * P + fill[jj_pick]
            fill[jj_pick] += 1
            slots[jj_pick] += 1
        newloc_of[base:base + npc] = newpos

    tloc = newloc_of[t]
    j = tloc // P
    tin = (tloc % P).astype(np.float64)

    # ---- per-core stream construction (pair units) ----
    # Pass 1: per core, order edges by (tile, is_repeat, orig pos);
    # compute first-use perm with per-run even alignment; count pairs.
    per_core_raw = []
    pairs_per_run = np.zeros((N_CORES, n_tiles), dtype=np.int64)
    tbl_rows = np.zeros(N_CORES, dtype=np.int64)
    for c in range(N_CORES):
        sel = np.nonzero(core == c)[0]
        sel = sel[np.argsort(j[sel], kind="stable")]  # tile-major
        hs = h[sel]
        js = j[sel]
        # first occurrence of each source in this stream order
        _, first_idx = np.unique(hs, return_index=True)
        is_first = np.zeros(len(sel), dtype=bool)
        is_first[first_idx] = True
        # reorder within run: firsts first (stable)
        o2 = np.argsort(js * 2 + (~is_first), kind="stable")
        hs, js, is_first = hs[o2], js[o2], is_first[o2]
        sel = sel[o2]
        # per-run first/repeat counts
        f_cnt = np.bincount(js[is_first], minlength=n_tiles)
        r_cnt = np.bincount(js[~is_first], minlength=n_tiles)
        pairs_per_run[c] = (f_cnt + 1) // 2 + r_cnt
        # x2p row of each first (per-run even-aligned cumsum)
        run_base = np.zeros(n_tiles + 1, dtype=np.int64)
        for jj in range(n_tiles):
            run_base[jj + 1] = run_base[jj] + f_cnt[jj] + (f_cnt[jj] & 1)
        tbl_rows[c] = run_base[n_tiles]
        # perm[source] = x2p row (within-run rank of firsts)
        firsts_pos = np.nonzero(is_first)[0]
        within = np.zeros(len(firsts_pos), dtype=np.int64)
        prev = -1
        k = 0
        for idx3 in range(len(firsts_pos)):
            jj = js[firsts_pos[idx3]]
            if jj != prev:
                k = 0
                prev = jj
            within[idx3] = k
            k += 1
        perm = np.full(n, -1, dtype=np.int64)
        perm[hs[firsts_pos]] = run_base[js[firsts_pos]] + within
        per_core_raw.append((sel, hs, js, is_first, f_cnt, r_cnt,
                             run_base, perm))

    run_pairs = pairs_per_run.max(axis=0)  # shared schedule
    n_blocks = -(-n_tiles // TILE_BLOCK)
    run_start = np.zeros(n_tiles, dtype=np.int64)  # in pairs
    pos = 0
    for jj in range(n_tiles):
        run_start[jj] = pos
        pos += int(run_pairs[jj])
    e_pairs = -(-pos // P) * P
    n_pcols = e_pairs // P
    NTBL = int(tbl_rows.max())
    NTBL += NTBL & 1
    n_tblpairs = NTBL // 2
    assert n_tblpairs <= 32767

    # one-hot columns: mmcol = 2*pc + h per (pair-col, half, tile)
    ohcols = []
    tile_ohcols = [[] for _ in range(n_tiles)]
    blk_ohranges = []
    for blk in range(n_blocks):
        oh0 = len(ohcols)
        for jj in range(blk * TILE_BLOCK,
                        min((blk + 1) * TILE_BLOCK, n_tiles)):
            s, r = int(run_start[jj]), int(run_pairs[jj])
            if r == 0:
                continue
            for pc in range(s // P, (s + r - 1) // P + 1):
                for hh in (0, 1):
                    tile_ohcols[jj].append((len(ohcols), 2 * pc + hh))
                    ohcols.append((pc, hh, jj))
        blk_ohranges.append((oh0, len(ohcols)))
    n_oh = len(ohcols)
    max_blk_oh = max(b1 - b0 for (b0, b1) in blk_ohranges)

    first_blk_of_pcol = np.full(n_pcols, n_blocks, dtype=np.int64)
    for (pc, hh, jj) in ohcols:
        first_blk_of_pcol[pc] = min(first_blk_of_pcol[pc],
                                    jj // TILE_BLOCK)

    # gidx slices + gather chunks over pair space
    slices = []
    per = -(-(e_pairs // N_SLICES) // P) * P
    lo = 0
    while lo < e_pairs:
        hi = min(lo + per, e_pairs)
        slices.append((lo, hi))
        lo = hi
    gathers = []  # (slice_k, s0, ln, fc) in pairs
    for k, (lo, hi) in enumerate(slices):
        c0 = lo
        while c0 < hi:
            ln = min(GATHER_CHUNK, hi - c0)
            fc = int(first_blk_of_pcol[c0 // P:(c0 + ln) // P].min())
            gathers.append((k, c0, ln, fc))
            c0 += ln
    gathers.sort(key=lambda g: (g[3], g[1]))

    # ---- per-core tensors ----
    import ml_dtypes
    per_core = []
    for c in range(N_CORES):
        (sel, hs, js, is_first, f_cnt, r_cnt, run_base,
         perm) = per_core_raw[c]
        # pair idx + slot meta
        gi = np.zeros(e_pairs, dtype=np.int16)          # pair idx
        tf = np.full(2 * e_pairs, -1.0, dtype=np.float64)
        town = np.full(2 * e_pairs, -1, dtype=np.int64)
        tin_c = tin[sel]
        # firsts: per run, slot k -> pair run_start+k/2, half k%2,
        # x2p row run_base+k
        fpos = np.nonzero(is_first)[0]
        rpos = np.nonzero(~is_first)[0]
        # within-run ranks
        def ranks(pos_arr, runs_arr):
            out = np.zeros(len(pos_arr), dtype=np.int64)
            prev = -1
            k = 0
            for i3 in range(len(pos_arr)):
                jj = runs_arr[i3]
                if jj != prev:
                    k = 0
                    prev = jj
                out[i3] = k
                k += 1
            return out
        frank = ranks(fpos, js[fpos])
        rrank = ranks(rpos, js[rpos])
        # firsts
        q_f = run_start[js[fpos]] + frank // 2
        h_f = frank % 2
        slot_f = 2 * q_f + h_f
        tf[slot_f] = tin_c[fpos]
        town[slot_f] = js[fpos]
        # pair idx for first-pairs: row/2 where row = run_base + 2*(pair offset)
        gi[q_f] = ((run_base[js[fpos]] + frank) // 2).astype(np.int16)
        # repeats: pair slot after the first-pairs of the run
        fp = (f_cnt + 1) // 2
        q_r = run_start[js[rpos]] + fp[js[rpos]] + rrank
        p_r = perm[hs[rpos]]
        h_r = p_r % 2
        slot_r = 2 * q_r + h_r
        tf[slot_r] = tin_c[rpos]
        town[slot_r] = js[rpos]
        gi[q_r] = (p_r // 2).astype(np.int16)

        # x2p table
        x2p = np.zeros((NTBL, d), dtype=np.float64)
        used = perm >= 0
        x2p[perm[used]] = x2[np.nonzero(used)[0]]
        x2p = x2p.reshape(NTBL // 2, 2 * d).astype(ml_dtypes.bfloat16)

        # wrap pair idx: [16, e/16] tiled x8 -> [128, e/16]
        wrap = np.tile(gi.reshape(e_pairs // 16, 16).T,
                       (8, 1)).astype(np.int16)

        # meta[p, k] for ohcol k=(pc, hh, tile)
        tf3 = tf.reshape(n_pcols, P, 2)      # [pc, partition, half]
        town3 = town.reshape(n_pcols, P, 2)
        pcarr = np.array([pc for (pc, _h, _j) in ohcols])
        hharr = np.array([_h for (_pc, _h, _j) in ohcols])
        jarr = np.array([_j for (_pc, _h, _j) in ohcols])
        m3 = np.where(town3[pcarr, :, hharr] == jarr[:, None],
                      tf3[pcarr, :, hharr], -1.0)  # [n_oh, 128]
        meta = m3.T.astype(ml_dtypes.bfloat16).copy()

        dnode = np.zeros(n_tiles * P, dtype=np.float32)
        nl = newloc_of[c * npc:(c + 1) * npc]
        dnode[nl] = dis[c * npc:(c + 1) * npc].astype(np.float32)
        dis_t = dnode.reshape(n_tiles, P).T.copy()

        per_core.append({"gidx": wrap, "meta": meta, "dis": dis_t,
                         "x2p": x2p})

    iota = np.tile(np.arange(P, dtype=np.float32), (P, 1))

    sched = {
        "n": n, "d": d, "npc": npc, "n_tiles": n_tiles,
        "e_pairs": e_pairs, "n_tblpairs": n_tblpairs,
        "n_blocks": n_blocks, "gathers": gathers, "slices": slices,
        "ohcols": ohcols, "tile_ohcols": tile_ohcols,
        "blk_ohranges": blk_ohranges, "n_oh": n_oh,
        "max_blk_oh": max_blk_oh, "newloc_of": newloc_of,
    }
    return sched, per_core, iota


def _build_program(sched):
    n, d, npc = sched["n"], sched["d"], sched["npc"]
    n_tiles, e_pairs = sched["n_tiles"], sched["e_pairs"]
    n_tblpairs = sched["n_tblpairs"]
    n_blocks, gathers = sched["n_blocks"], sched["gathers"]
    tile_ohcols = sched["tile_ohcols"]
    blk_ohranges = sched["blk_ohranges"]
    n_oh, max_blk_oh = sched["n_oh"], sched["max_blk_oh"]
    slices = sched["slices"]

    nc = bacc.Bacc("TRN2", target_bir_lowering=False, debug=False,
                   num_devices=N_CORES, num_swdge_queues=4,
                   dynamic_dma_scratch_size=98304)
    f32 = mybir.dt.float32
    bf16 = mybir.dt.bfloat16
    x_d = nc.dram_tensor("x2p", [n_tblpairs, 2 * d], bf16,
                         kind="ExternalInput")
    iota_d = nc.dram_tensor("iota", [P, P], f32, kind="ExternalInput")
    gidx_d = nc.dram_tensor("gidx", [P, e_pairs // 16], mybir.dt.int16,
                            kind="ExternalInput")
    meta_d = nc.dram_tensor("meta", [P, n_oh], bf16, kind="ExternalInput")
    dis_d = nc.dram_tensor("dis", [P, n_tiles], f32, kind="ExternalInput")
    y_d = nc.dram_tensor("y", [npc, d], f32, kind="ExternalOutput")

    nc.gpsimd.load_library(mlp_lib)

    gather_of_pcol = {}
    for gid, (sk, s0, ln, _fc) in enumerate(gathers):
        for pc in range(s0 // P, (s0 + ln) // P):
            gather_of_pcol[pc] = (gid, s0 // P)

    relu = mybir.ActivationFunctionType.Relu

    with tile.TileContext(nc) as tc:
        with (
            tc.tile_pool(name="const", bufs=1) as cpool,
            tc.tile_pool(name="gather", bufs=GPOOL_BUFS) as gpool,
            tc.tile_pool(name="onehot", bufs=OPOOL_BUFS) as opool,
            tc.tile_pool(name="psum", bufs=7, space="PSUM") as ppool,
            tc.tile_pool(name="psiota", bufs=1, space="PSUM") as ipool,
            tc.tile_pool(name="outs", bufs=6) as ypool,
        ):
            gidx_tiles = {}
            for k, (s0, s1) in enumerate(slices):
                gix = cpool.tile([P, (s1 - s0) // 16], mybir.dt.int16,
                                 tag=f"gidx{k}")
                nc.sync.dma_start(gix[:], gidx_d[:, s0 // 16:s1 // 16])
                gidx_tiles[k] = (gix, s0)
                if k == 0:
                    iota_t = cpool.tile([P, P], f32, tag="iota")
                    nc.sync.dma_start(iota_t[:], iota_d[:, :])
                    meta_t = cpool.tile([P, n_oh], bf16, tag="meta")
                    nc.sync.dma_start(meta_t[:], meta_d[:, :])
                    dis_t = cpool.tile([P, n_tiles], f32, tag="dis")
                    nc.sync.dma_start(dis_t[:], dis_d[:, :])

            iota_p = ipool.tile([P, P], f32, tag="iop")
            nc.vector.tensor_copy(iota_p[:], iota_t[:])

            # last block's one-hots depend only on meta: build up front
            lo0, lo1 = blk_ohranges[n_blocks - 1]
            nlo = lo1 - lo0
            last_oh = cpool.tile([P, nlo * P], mybir.dt.float8e4, tag="lastoh")
            nc.vector.tensor_tensor(
                last_oh[:, :nlo * P].rearrange("p (c f) -> p c f", f=P),
                iota_p[:, None, :].broadcast_to([P, nlo, P]),
                meta_t[:, lo0:lo1, None].broadcast_to([P, nlo, P]),
                mybir.AluOpType.is_equal,
            )

            gtiles = {}

            def issue_gather(gid):
                sk, s0, ln, _fc = gathers[gid]
                gt = gpool.tile([P, (GATHER_CHUNK // P) * 2 * d], bf16,
                                tag="gt", name=f"gt{gid}")
                ncols_g = ln // P
                gt_3d = gt[:, :ncols_g * 2 * d].rearrange(
                    "p (c d) -> p c d", d=2 * d)
                gix, sl0 = gidx_tiles[sk]
                nc.gpsimd.dma_gather(
                    gt_3d,
                    x_d[:, :],
                    gix[:, (s0 - sl0) // 16:(s0 + ln - sl0) // 16],
                    ln, ln, 2 * d,
                    single_packet=False,
                    queue_num=gid % 4,
                )
                gtiles[gid] = gt

            next_gather = 0
            for blk in range(n_blocks):
                tiles_blk = range(blk * TILE_BLOCK,
                                  min((blk + 1) * TILE_BLOCK, n_tiles))
                while next_gather < len(gathers):
                    if gathers[next_gather][3] > blk + 1:
                        break
                    issue_gather(next_gather)
                    next_gather += 1

                oh0, oh1 = blk_ohranges[blk]
                nbo = oh1 - oh0
                if blk == n_blocks - 1:
                    ohblk = last_oh
                else:
                    ohblk = opool.tile([P, max_blk_oh * P], mybir.dt.float8e4,
                                       tag="ohb", name=f"ohb{blk}")
                    nc.vector.tensor_tensor(
                        ohblk[:, :nbo * P].rearrange(
                            "p (c f) -> p c f", f=P),
                        iota_p[:, None, :].broadcast_to([P, nbo, P]),
                        meta_t[:, oh0:oh1, None].broadcast_to([P, nbo, P]),
                        mybir.AluOpType.is_equal,
                    )

                for jj in tiles_blk:
                    ohlist = tile_ohcols[jj]
                    rows = min(P, npc - jj * P)
                    yt = ypool.tile([P, d], f32, tag="yt", name=f"yt{jj}")
                    pt = ppool.tile([P, d], f32, tag="ps", name=f"ps{jj}")
                    for si, (ohk, col2) in enumerate(ohlist):
                        pc, hh = col2 // 2, col2 % 2
                        gid, pc0 = gather_of_pcol[pc]
                        gt = gtiles[gid]
                        off = (pc - pc0) * 2 * d + hh * d
                        nc.tensor.matmul(
                            pt[:],
                            lhsT=ohblk[:, (ohk - oh0) * P:
                                       (ohk - oh0 + 1) * P],
                            rhs=gt[:, off:off + d],
                            start=(si == 0),
                            stop=(si == len(ohlist) - 1),
                        )
                    nc.scalar.activation(yt[:], pt[:], relu,
                                         scale=dis_t[:, jj:jj + 1])
                    nc.sync.dma_start(y_d[jj * P:jj * P + rows, :],
                                      yt[:rows, :])

    nc.compile()
    return nc


def _run(x, h, t, trace=False):
    import time
    t0 = time.monotonic()
    sched, per_core, iota = _preprocess(np.asarray(x), np.asarray(h),
                                        np.asarray(t))
    t1 = time.monotonic()
    print(f"[kernel] preprocess {t1 - t0:.1f}s  e_pairs={sched['e_pairs']} "
          f"pcols={sched['e_pairs'] // P} oh={sched['n_oh']} "
          f"tblpairs={sched['n_tblpairs']} "
          f"gathers={len(sched['gathers'])}", flush=True)
    nc = _build_program(sched)
    t2 = time.monotonic()
    print(f"[kernel] build {t2 - t1:.1f}s", flush=True)
    in_maps = [
        {"x2p": pc["x2p"], "iota": iota, "gidx": pc["gidx"],
         "meta": pc["meta"], "dis": pc["dis"]}
        for pc in per_core
    ]
    res = run_bass_kernel_spmd(nc, in_maps, core_ids=list(range(N_CORES)),
                               trace=trace)
    t3 = time.monotonic()
    print(f"[kernel] compile+run {t3 - t2:.1f}s", flush=True)
    ycat = np.concatenate([res.results[c]["y"] for c in range(N_CORES)],
                          axis=0)
    npc = sched["npc"]
    newloc = sched["newloc_of"]
    src = (np.arange(len(newloc)) // npc) * npc + newloc
    y = ycat[src]
    return y, res


def kernel(x, h, t):
    y, _ = _run(np.asarray(x), np.asarray(h), np.asarray(t))
    return y


# revision 60
# speedup vs baseline: 1.0380x; 1.0175x over previous
"""PlainGCN message passing on 8 TRN2 NeuronCores.

Computation (reference):
    deg = bincount(h); dis = deg**-0.5; norm = dis[t]*dis[h]
    out = relu(segment_sum(norm[:,None] * x[h], t, N))

Strategy (v18, paired descriptors; ~164us; v17 was ~200us, v13 231us):
  - Fold dis[h] into x host-side: x2 = dis[:,None]*x (bf16). Then
    out[t] = relu(dis[t] * segment_sum(x2[h], t)); dis[t] fused with
    the ReLU on ScalarE. Shard edges by destination: core c owns dest
    nodes [c*N/8, (c+1)*N/8).
  - The wall (v13-v17) is SWDGE descriptor generation: the Pool
    engine gather ucode is ~2.1 ns SERIAL per descriptor (4-queue
    batches; measured). Cost is per-DESCRIPTOR, not per-byte: 512B
    two-row descriptors cost ~2.12ns vs 2.01ns for 256B (measured)
    — so v18 makes most descriptors deliver TWO edge rows.
  - Per core, upload x2p: x2 rows permuted into stream-first-use
    order (distinct sources ~52.8k < 2*32767, so PAIR indices fit
    int16 over ONE window — no source buckets, one run per dest
    tile). Each tile's run puts first-occurrence slots first: their
    x2p rows are consecutive and even-aligned, so one elem_size=256
    descriptor covers TWO slots. Repeat slots (~30%) cost one
    descriptor each (used half = first-use parity, other half -1).
    Descriptors/core: ~76k -> ~50k.
  - Stream layout in PAIR units: runs back to back (run length = max
    over cores), end pad to 128 pairs. Stream slot s = (pair q=s/2,
    half h=s%2) -> gathered tile [128, cols, 256] at partition q%128,
    pair-col q/128, bytes h*256. Every pair slot is gathered (pads
    read row 0) — no memsets, no stale SBUF.
  - Segment-sum: one one-hot per (pair-col, half, tile) triple via
    is_equal(iota bf16-meta) with FP8E4 output (0/1 exact; halves
    one-hot SBUF — spent on deeper pools; DVE build speed unchanged,
    it is elementwise-bound). fp8 lhsT x bf16 rhs matmul verified
    exact on HW. Whole block's one-hots in one DVE tensor_tensor
    (iota in PSUM: single SBUF port, doesn't stall Q7 desc-gen).
    TensorE accumulates into PSUM; Relu(dis_t*psum) on ScalarE; DMA
    out. Last block's one-hots build up front.
  - 1024 (pair-)idxs is the hard ucode cap per dma_gather;
    single_packet=False; 4 SWDGE queues; deep FIFO + pools.
  - Remaining profile (~168us trace): gathers end ~140us (51 chunks,
    ~8.8us per 4-chunk batch + some stalls), PE matmuls lag ~21us
    behind (964 matmuls), DVE one-hot build 132us busy — PE/DVE are
    now co-bottlenecks with the stream. Next levers: prune
    all-empty (pc,half,tile) one-hot columns (union across cores),
    smooth late-block matmul bunching, trim the 17us lib-load ramp.
  - Device quirk: intermittent ~12.6% slower GpSimd clock mode
    (persists across runs); NEURON_RT_RESET_CORES=1 clears it.
"""

import numpy as np

import concourse.bacc as bacc
import concourse.mybir as mybir
import concourse.tile as tile
from concourse.bass_utils import run_bass_kernel_spmd
from concourse.library_config import mlp as mlp_lib

P = 128
N_NODES = 100000
D_FEAT = 128
N_CORES = 8
TILE_BLOCK = 4       # dest tiles per block
GATHER_CHUNK = 1024  # max pair-idxs per dma_gather
GPOOL_BUFS = 16
OPOOL_BUFS = 4
N_SLICES = 4


def _preprocess(x, h, t):
    n, d = x.shape
    assert (n, d) == (N_NODES, D_FEAT)
    npc = n // N_CORES
    n_tiles = -(-npc // P)

    h = h.astype(np.int64)
    t = t.astype(np.int64)

    deg = np.bincount(h, minlength=n).astype(np.float64)
    dis = np.where(deg > 0, deg, 1.0) ** -0.5
    x2 = (x.astype(np.float64) * dis[:, None]).astype(np.float32)

    core = t // npc

    # Balanced node->tile assignment per core: flatten per-tile edge
    # counts so max-over-cores run lengths stay near the mean. Last
    # two blocks biased light so the stream tail is small.
    tloc_orig = t - core * npc
    newloc_of = np.zeros(n, dtype=np.int64)
    for c in range(N_CORES):
        base = c * npc
        vdeg = np.zeros(npc, dtype=np.int64)
        selc = np.nonzero(core == c)[0]
        np.add.at(vdeg, tloc_orig[selc], 1)
        order = np.argsort(-vdeg, kind="stable")
        loads = np.zeros(n_tiles, dtype=np.int64)
        slots = np.zeros(n_tiles, dtype=np.int64)
        fill = np.zeros(n_tiles, dtype=np.int64)
        cap = np.full(n_tiles, P, dtype=np.int64)
        cap[n_tiles - 1] = npc - (n_tiles - 1) * P
        bias = np.zeros(n_tiles, dtype=np.int64)
        lastblk0 = ((n_tiles - 1) // TILE_BLOCK) * TILE_BLOCK
        bias[lastblk0:] = 64
        if lastblk0 >= TILE_BLOCK:
            bias[lastblk0 - TILE_BLOCK:lastblk0] = 32
        newpos = np.zeros(npc, dtype=np.int64)
        for i in order:
            score = loads + bias
            score[slots >= cap] = 1 << 60
            jj_pick = int(np.argmin(score))
            loads[jj_pick] += vdeg[i]
            newpos[i] = jj_pick * P + fill[jj_pick]
            fill[jj_pick] += 1
            slots[jj_pick] += 1
        newloc_of[base:base + npc] = newpos

    tloc = newloc_of[t]
    j = tloc // P
    tin = (tloc % P).astype(np.float64)

    # ---- per-core stream construction (pair units) ----
    # Pass 1: per core, order edges by (tile, is_repeat, orig pos);
    # compute first-use perm with per-run even alignment; count pairs.
    per_core_raw = []
    pairs_per_run = np.zeros((N_CORES, n_tiles), dtype=np.int64)
    tbl_rows = np.zeros(N_CORES, dtype=np.int64)
    for c in range(N_CORES):
        sel = np.nonzero(core == c)[0]
        sel = sel[np.argsort(j[sel], kind="stable")]  # tile-major
        hs = h[sel]
        js = j[sel]
        # first occurrence of each source in this stream order
        _, first_idx = np.unique(hs, return_index=True)
        is_first = np.zeros(len(sel), dtype=bool)
        is_first[first_idx] = True
        # reorder within run: firsts first (stable)
        o2 = np.argsort(js * 2 + (~is_first), kind="stable")
        hs, js, is_first = hs[o2], js[o2], is_first[o2]
        sel = sel[o2]
        # per-run first/repeat counts
        f_cnt = np.bincount(js[is_first], minlength=n_tiles)
        r_cnt = np.bincount(js[~is_first], minlength=n_tiles)
        pairs_per_run[c] = (f_cnt + 1) // 2 + r_cnt
        # x2p row of each first (per-run even-aligned cumsum)
        run_base = np.zeros(n_tiles + 1, dtype=np.int64)
        for jj in range(n_tiles):
            run_base[jj + 1] = run_base[jj] + f_cnt[jj] + (f_cnt[jj] & 1)
        tbl_rows[c] = run_base[n_tiles]
        # perm[source] = x2p row (within-run rank of firsts)
        firsts_pos = np.nonzero(is_first)[0]
        within = np.zeros(len(firsts_pos), dtype=np.int64)
        prev = -1
        k = 0
        for idx3 in range(len(firsts_pos)):
            jj = js[firsts_pos[idx3]]
            if jj != prev:
                k = 0
                prev = jj
            within[idx3] = k
            k += 1
        perm = np.full(n, -1, dtype=np.int64)
        perm[hs[firsts_pos]] = run_base[js[firsts_pos]] + within
        per_core_raw.append((sel, hs, js, is_first, f_cnt, r_cnt,
                             run_base, perm))

    run_pairs = pairs_per_run.max(axis=0)  # shared schedule
    n_blocks = -(-n_tiles // TILE_BLOCK)
    run_start = np.zeros(n_tiles, dtype=np.int64)  # in pairs
    pos = 0
    for jj in range(n_tiles):
        run_start[jj] = pos
        pos += int(run_pairs[jj])
    e_pairs = -(-pos // P) * P
    n_pcols = e_pairs // P
    NTBL = int(tbl_rows.max())
    NTBL += NTBL & 1
    n_tblpairs = NTBL // 2
    assert n_tblpairs <= 32767

    # one-hot columns: mmcol = 2*pc + h per (pair-col, half, tile)
    ohcols = []
    tile_ohcols = [[] for _ in range(n_tiles)]
    blk_ohranges = []
    for blk in range(n_blocks):
        oh0 = len(ohcols)
        for jj in range(blk * TILE_BLOCK,
                        min((blk + 1) * TILE_BLOCK, n_tiles)):
            s, r = int(run_start[jj]), int(run_pairs[jj])
            if r == 0:
                continue
            for pc in range(s // P, (s + r - 1) // P + 1):
                for hh in (0, 1):
                    tile_ohcols[jj].append((len(ohcols), 2 * pc + hh))
                    ohcols.append((pc, hh, jj))
        blk_ohranges.append((oh0, len(ohcols)))
    n_oh = len(ohcols)
    max_blk_oh = max(b1 - b0 for (b0, b1) in blk_ohranges)

    first_blk_of_pcol = np.full(n_pcols, n_blocks, dtype=np.int64)
    for (pc, hh, jj) in ohcols:
        first_blk_of_pcol[pc] = min(first_blk_of_pcol[pc],
                                    jj // TILE_BLOCK)

    # gidx slices + gather chunks over pair space
    slices = []
    per = -(-(e_pairs // N_SLICES) // P) * P
    lo = 0
    while lo < e_pairs:
        hi = min(lo + per, e_pairs)
        slices.append((lo, hi))
        lo = hi
    gathers = []  # (slice_k, s0, ln, fc) in pairs
    for k, (lo, hi) in enumerate(slices):
        c0 = lo
        while c0 < hi:
            ln = min(GATHER_CHUNK, hi - c0)
            fc = int(first_blk_of_pcol[c0 // P:(c0 + ln) // P].min())
            gathers.append((k, c0, ln, fc))
            c0 += ln
    gathers.sort(key=lambda g: (g[3], g[1]))

    # ---- per-core tensors ----
    import ml_dtypes
    per_core = []
    for c in range(N_CORES):
        (sel, hs, js, is_first, f_cnt, r_cnt, run_base,
         perm) = per_core_raw[c]
        # pair idx + slot meta
        gi = np.zeros(e_pairs, dtype=np.int16)          # pair idx
        tf = np.full(2 * e_pairs, -1.0, dtype=np.float64)
        town = np.full(2 * e_pairs, -1, dtype=np.int64)
        tin_c = tin[sel]
        # firsts: per run, slot k -> pair run_start+k/2, half k%2,
        # x2p row run_base+k
        fpos = np.nonzero(is_first)[0]
        rpos = np.nonzero(~is_first)[0]
        # within-run ranks
        def ranks(pos_arr, runs_arr):
            out = np.zeros(len(pos_arr), dtype=np.int64)
            prev = -1
            k = 0
            for i3 in range(len(pos_arr)):
                jj = runs_arr[i3]
                if jj != prev:
                    k = 0
                    prev = jj
                out[i3] = k
                k += 1
            return out
        frank = ranks(fpos, js[fpos])
        rrank = ranks(rpos, js[rpos])
        # firsts
        q_f = run_start[js[fpos]] + frank // 2
        h_f = frank % 2
        slot_f = 2 * q_f + h_f
        tf[slot_f] = tin_c[fpos]
        town[slot_f] = js[fpos]
        # pair idx for first-pairs: row/2 where row = run_base + 2*(pair offset)
        gi[q_f] = ((run_base[js[fpos]] + frank) // 2).astype(np.int16)
        # repeats: pair slot after the first-pairs of the run
        fp = (f_cnt + 1) // 2
        q_r = run_start[js[rpos]] + fp[js[rpos]] + rrank
        p_r = perm[hs[rpos]]
        h_r = p_r % 2
        slot_r = 2 * q_r + h_r
        tf[slot_r] = tin_c[rpos]
        town[slot_r] = js[rpos]
        gi[q_r] = (p_r // 2).astype(np.int16)

        # x2p table
        x2p = np.zeros((NTBL, d), dtype=np.float64)
        used = perm >= 0
        x2p[perm[used]] = x2[np.nonzero(used)[0]]
        x2p = x2p.reshape(NTBL // 2, 2 * d).astype(ml_dtypes.bfloat16)

        # wrap pair idx: [16, e/16] tiled x8 -> [128, e/16]
        wrap = np.tile(gi.reshape(e_pairs // 16, 16).T,
                       (8, 1)).astype(np.int16)

        # meta[p, k] for ohcol k=(pc, hh, tile)
        tf3 = tf.reshape(n_pcols, P, 2)      # [pc, partition, half]
        town3 = town.reshape(n_pcols, P, 2)
        pcarr = np.array([pc for (pc, _h, _j) in ohcols])
        hharr = np.array([_h for (_pc, _h, _j) in ohcols])
        jarr = np.array([_j for (_pc, _h, _j) in ohcols])
        m3 = np.where(town3[pcarr, :, hharr] == jarr[:, None],
                      tf3[pcarr, :, hharr], -1.0)  # [n_oh, 128]
        meta = m3.T.astype(ml_dtypes.bfloat16).copy()

        dnode = np.zeros(n_tiles * P, dtype=np.float32)
        nl = newloc_of[c * npc:(c + 1) * npc]
        dnode[nl] = dis[c * npc:(c + 1) * npc].astype(np.float32)
        dis_t = dnode.reshape(n_tiles, P).T.copy()

        per_core.append({"gidx": wrap, "meta": meta, "dis": dis_t,
                         "x2p": x2p})

    iota = np.tile(np.arange(P, dtype=np.float32), (P, 1))

    sched = {
        "n": n, "d": d, "npc": npc, "n_tiles": n_tiles,
        "e_pairs": e_pairs, "n_tblpairs": n_tblpairs,
        "n_blocks": n_blocks, "gathers": gathers, "slices": slices,
        "ohcols": ohcols, "tile_ohcols": tile_ohcols,
        "blk_ohranges": blk_ohranges, "n_oh": n_oh,
        "max_blk_oh": max_blk_oh, "newloc_of": newloc_of,
    }
    return sched, per_core, iota


def _build_program(sched):
    n, d, npc = sched["n"], sched["d"], sched["npc"]
    n_tiles, e_pairs = sched["n_tiles"], sched["e_pairs"]
    n_tblpairs = sched["n_tblpairs"]
    n_blocks, gathers = sched["n_blocks"], sched["gathers"]
    tile_ohcols = sched["tile_ohcols"]
    blk_ohranges = sched["blk_ohranges"]
    n_oh, max_blk_oh = sched["n_oh"], sched["max_blk_oh"]
    slices = sched["slices"]

    nc = bacc.Bacc("TRN2", target_bir_lowering=False, debug=False,
                   num_devices=N_CORES, num_swdge_queues=4,
                   dynamic_dma_scratch_size=98304)
    f32 = mybir.dt.float32
    bf16 = mybir.dt.bfloat16
    x_d = nc.dram_tensor("x2p", [n_tblpairs, 2 * d], bf16,
                         kind="ExternalInput")
    iota_d = nc.dram_tensor("iota", [P, P], f32, kind="ExternalInput")
    gidx_d = nc.dram_tensor("gidx", [P, e_pairs // 16], mybir.dt.int16,
                            kind="ExternalInput")
    meta_d = nc.dram_tensor("meta", [P, n_oh], bf16, kind="ExternalInput")
    dis_d = nc.dram_tensor("dis", [P, n_tiles], f32, kind="ExternalInput")
    y_d = nc.dram_tensor("y", [npc, d], f32, kind="ExternalOutput")

    nc.gpsimd.load_library(mlp_lib)

    gather_of_pcol = {}
    for gid, (sk, s0, ln, _fc) in enumerate(gathers):
        for pc in range(s0 // P, (s0 + ln) // P):
            gather_of_pcol[pc] = (gid, s0 // P)

    relu = mybir.ActivationFunctionType.Relu

    with tile.TileContext(nc) as tc:
        with (
            tc.tile_pool(name="const", bufs=1) as cpool,
            tc.tile_pool(name="gather", bufs=GPOOL_BUFS) as gpool,
            tc.tile_pool(name="onehot", bufs=OPOOL_BUFS) as opool,
            tc.tile_pool(name="psum", bufs=7, space="PSUM") as ppool,
            tc.tile_pool(name="psiota", bufs=1, space="PSUM") as ipool,
            tc.tile_pool(name="outs", bufs=6) as ypool,
        ):
            gidx_tiles = {}
            for k, (s0, s1) in enumerate(slices):
                gix = cpool.tile([P, (s1 - s0) // 16], mybir.dt.int16,
                                 tag=f"gidx{k}")
                nc.sync.dma_start(gix[:], gidx_d[:, s0 // 16:s1 // 16])
                gidx_tiles[k] = (gix, s0)
                if k == 0:
                    iota_t = cpool.tile([P, P], f32, tag="iota")
                    nc.sync.dma_start(iota_t[:], iota_d[:, :])
                    meta_t = cpool.tile([P, n_oh], bf16, tag="meta")
                    nc.sync.dma_start(meta_t[:], meta_d[:, :])
                    dis_t = cpool.tile([P, n_tiles], f32, tag="dis")
                    nc.sync.dma_start(dis_t[:], dis_d[:, :])

            iota_p = ipool.tile([P, P], f32, tag="iop")
            nc.vector.tensor_copy(iota_p[:], iota_t[:])

            # last block's one-hots depend only on meta: build up front
            lo0, lo1 = blk_ohranges[n_blocks - 1]
            nlo = lo1 - lo0
            last_oh = cpool.tile([P, nlo * P], mybir.dt.float8e4, tag="lastoh")
            nc.vector.tensor_tensor(
                last_oh[:, :nlo * P].rearrange("p (c f) -> p c f", f=P),
                iota_p[:, None, :].broadcast_to([P, nlo, P]),
                meta_t[:, lo0:lo1, None].broadcast_to([P, nlo, P]),
                mybir.AluOpType.is_equal,
            )

            gtiles = {}

            def issue_gather(gid):
                sk, s0, ln, _fc = gathers[gid]
                gt = gpool.tile([P, (GATHER_CHUNK // P) * 2 * d], bf16,
                                tag="gt", name=f"gt{gid}")
                ncols_g = ln // P
                gt_3d = gt[:, :ncols_g * 2 * d].rearrange(
                    "p (c d) -> p c d", d=2 * d)
                gix, sl0 = gidx_tiles[sk]
                nc.gpsimd.dma_gather(
                    gt_3d,
                    x_d[:, :],
                    gix[:, (s0 - sl0) // 16:(s0 + ln - sl0) // 16],
                    ln, ln, 2 * d,
                    single_packet=False,
                    queue_num=gid % 4,
                )
                gtiles[gid] = gt

            next_gather = 0
            for blk in range(n_blocks):
                tiles_blk = range(blk * TILE_BLOCK,
                                  min((blk + 1) * TILE_BLOCK, n_tiles))
                while next_gather < len(gathers):
                    if gathers[next_gather][3] > blk + 1:
                        break
                    issue_gather(next_gather)
                    next_gather += 1

                oh0, oh1 = blk_ohranges[blk]
                nbo = oh1 - oh0
                if blk == n_blocks - 1:
                    ohblk = last_oh
                else:
                    ohblk = opool.tile([P, max_blk_oh * P], mybir.dt.float8e4,
                                       tag="ohb", name=f"ohb{blk}")
                    nc.vector.tensor_tensor(
                        ohblk[:, :nbo * P].rearrange(
                            "p (c f) -> p c f", f=P),
                        iota_p[:, None, :].broadcast_to([P, nbo, P]),
                        meta_t[:, oh0:oh1, None].broadcast_to([P, nbo, P]),
                        mybir.AluOpType.is_equal,
                    )

                for jj in tiles_blk:
                    ohlist = tile_ohcols[jj]
                    rows = min(P, npc - jj * P)
                    yt = ypool.tile([P, d], f32, tag="yt", name=f"yt{jj}")
                    pt = ppool.tile([P, d], f32, tag="ps", name=f"ps{jj}")
                    for si, (ohk, col2) in enumerate(ohlist):
                        pc, hh = col2 // 2, col2 % 2
                        gid, pc0 = gather_of_pcol[pc]
                        gt = gtiles[gid]
                        off = (pc - pc0) * 2 * d + hh * d
                        nc.tensor.matmul(
                            pt[:],
                            lhsT=ohblk[:, (ohk - oh0) * P:
                                       (ohk - oh0 + 1) * P],
                            rhs=gt[:, off:off + d],
                            start=(si == 0),
                            stop=(si == len(ohlist) - 1),
                        )
                    nc.scalar.activation(yt[:], pt[:], relu,
                                         scale=dis_t[:, jj:jj + 1])
                    nc.sync.dma_start(y_d[jj * P:jj * P + rows, :],
                                      yt[:rows, :])

    nc.compile()
    return nc


def _run(x, h, t, trace=False):
    import time
    t0 = time.monotonic()
    sched, per_core, iota = _preprocess(np.asarray(x), np.asarray(h),
                                        np.asarray(t))
    t1 = time.monotonic()
    print(f"[kernel] preprocess {t1 - t0:.1f}s  e_pairs={sched['e_pairs']} "
          f"pcols={sched['e_pairs'] // P} oh={sched['n_oh']} "
          f"tblpairs={sched['n_tblpairs']} "
          f"gathers={len(sched['gathers'])}", flush=True)
    nc = _build_program(sched)
    t2 = time.monotonic()
    print(f"[kernel] build {t2 - t1:.1f}s", flush=True)
    in_maps = [
        {"x2p": pc["x2p"], "iota": iota, "gidx": pc["gidx"],
         "meta": pc["meta"], "dis": pc["dis"]}
        for pc in per_core
    ]
    res = run_bass_kernel_spmd(nc, in_maps, core_ids=list(range(N_CORES)),
                               trace=trace)
    t3 = time.monotonic()
    print(f"[kernel] compile+run {t3 - t2:.1f}s", flush=True)
    ycat = np.concatenate([res.results[c]["y"] for c in range(N_CORES)],
                          axis=0)
    npc = sched["npc"]
    newloc = sched["newloc_of"]
    src = (np.arange(len(newloc)) // npc) * npc + newloc
    y = ycat[src]
    return y, res


def kernel(x, h, t):
    y, _ = _run(np.asarray(x), np.asarray(h), np.asarray(t))
    return y
